# revision 57
# baseline (speedup 1.0000x reference)
"""Trainium2 Bass kernel for DeepBayesianDKVMN (nn_DeepBayesianDKVMN_39857296507058).

Math restructuring
------------------
The reference's sequential Bayesian-write scan is *linear* in the memory
state: the per-step precision/evidence increments depend only on step-t
inputs, never on the evolving state.  So the scan collapses to exclusive
cumulative sums over time, and everything else is batch-parallel:

  - front end: q_table[q_ids] @ q2k_w @ key_embeds.T folds into ONE
    [NQ+1, M] "sim table" gather (host precomputes the table, device does
    a dma_gather of 256B rows).
  - softmax: logits = sim + bias[t,m]; exp(bias) is multiplied in as eb.
  - per-(b,t) evidence vector comb[b,t,:] is a rank-4 combination of four
    fixed V-vectors, and those four only ever enter through one fixed
    gp-weighted combination, so the [S,M,V] write-aggregation reduces to
    THREE [S,M] batch sums: colsum, P (precision), Hw (weighted evidence).
  - the 1/Z softmax normalizations fold into the per-(b,t) feature matrix
    F' = F * [1/Z, 1/Z, 1/Z^2], contracted against [ee | ee^2] by tiny
    per-t PE matmuls (contraction over the 128 batch rows on partitions).
  - the per-(t,m) logit-bias factor eb commutes out of every
    b-contraction, so it is never materialized per-(b,t,m): it scales the
    [M,S] stats (ebts), forms the Z matmul columns (ebblk), and folds
    into the read vector g.
  - AllReduce over the three [S,M] stats; exclusive cumsums via
    tensor_tensor_scan; read vector g[t,m]; preds = zr * (ee . g) + pred_b.

Performance structure
---------------------
  - the dma_gather descriptor stream is spread over all 4 SWDGE queues
    (4 sub-gathers x 16 chunks, round-robin) so four SDMA engines drain
    concurrently; single-queue drain was the original 512us bottleneck.
  - 16 chunks of 32 timesteps pipeline gather/exp/stats; nothing else is
    allowed to produce bulk SDMA traffic during the gather (HWDGE
    transposes / broadcast streams starve the SWDGE drains), so chunk
    transposes run on the PE (via identity matmuls) and PSUM drains on
    the scalar engine.
  - exp(sim) chunks are PE-transposed into a resident [2t x 64m, b]
    tensor; Z and the final read-dot are pair-matmuls against
    block-diagonal [128, 2] column slices (ebblk / gblk), landing in
    [b, t] layout with no output transpose.
  - the stat AllReduce is slice-rate bound on the CC stream
    (~0.69us/2048 elements + ~8us/op rendezvous), so it is split into 6
    groups fired as their chunks complete; only the last 1-chunk group is
    exposed past the gather.
  - phase C (cumsums -> g) and phase D (read-dot) are each split at
    t=384 with per-scan carries, overlapping the collective tail; dummy
    PE transposes bridge the idle gap so the HAM clock stays at 2.4GHz
    for the phase-D weight loads.

Sharding: batch 1024 -> 128 rows per core across 8 cores (data parallel),
as the sharding hint suggests; the all-reduce is the per-slot aggregated
evidence/precision, shrunk by the rank-4 trick.
"""

import numpy as np
from contextlib import ExitStack

import concourse.bass as bass
import concourse.tile as tile
from concourse import bacc, mybir
from concourse.bass_utils import run_bass_kernel_spmd

# problem dims (hardcoded per spec)
B, S, M, K, V, E, NQ, C = 1024, 512, 64, 64, 128, 64, 10000, 4
NCORES = 8
BL = B // NCORES            # 128 batch rows per core
TC = 32                     # timestep chunk
NCH = S // TC               # 16 chunks
NIDX = BL * TC              # gather indices per chunk = 4096
SPLIT = 4                   # sub-gathers per chunk (queue spreading)
NQUEUES = 4                 # SWDGE queues (max 4)
ARSPLIT = 12                # chunks in phase C/D's first (overlapped) part
AR_GROUP_ENDS = (4, 8, 12, 14, 15, 16)  # stat-collective group boundaries
F32 = mybir.dt.float32
F16 = mybir.dt.float16
I16 = mybir.dt.int16
GSCALE = 1024.0            # fp16 pre-scale for the tiny g values
ALU = mybir.AluOpType
AXT = mybir.AxisListType
ACTF = mybir.ActivationFunctionType

_CACHE = {}


def _build(single_core=False):
    nc = bacc.Bacc("TRN2", target_bir_lowering=False, debug=False,
                   num_devices=1 if single_core else NCORES,
                   num_swdge_queues=NQUEUES)

    t_simtab = nc.dram_tensor("simtab", [NQ + 1, M], F32, kind="ExternalInput")
    t_idx = nc.dram_tensor("idx16", [128, NCH * NIDX // 16], I16,
                           kind="ExternalInput")
    t_ftab = nc.dram_tensor("ftab", [BL, S, 3], F16, kind="ExternalInput")
    t_ebblk = nc.dram_tensor("ebblk", [128, S], F16, kind="ExternalInput")
    t_ebts = nc.dram_tensor("ebts", [M, S, 2], F16, kind="ExternalInput")
    t_sc = nc.dram_tensor("scal", [M, 8], F32, kind="ExternalInput")
    t_pb = nc.dram_tensor("pb", [BL, 1], F32, kind="ExternalInput")
    t_ident = nc.dram_tensor("ident", [128, 128], F16, kind="ExternalInput")
    t_preds = nc.dram_tensor("preds", [BL, S], F32, kind="ExternalOutput")

    with tile.TileContext(nc) as tc:
        _build_body(nc, tc, single_core, t_simtab, t_idx, t_ftab, t_ebblk,
                    t_ebts, t_sc, t_pb, t_ident, t_preds)
    nc.compile()
    return nc


def _build_body(nc, tc, single_core, t_simtab, t_idx, t_ftab, t_ebblk,
                t_ebts, t_sc, t_pb, t_ident, t_preds):
    with ExitStack() as ctx:
        cpool = ctx.enter_context(tc.tile_pool(name="const", bufs=1))
        dpool = ctx.enter_context(tc.tile_pool(name="dram", bufs=1,
                                               space="DRAM"))
        # internal DRAM: grouped stat buffers.  The CC stream costs
        # ~4.3us/op + 0.69us per 2048-element slice, so the AllReduce is
        # split into a few groups fired as their chunks complete; only the
        # last (small) group is exposed after the gather phase.
        garr = [0] + list(AR_GROUP_ENDS)
        d_hin = [dpool.tile([M, (garr[i + 1] - garr[i]) * TC * 3], F16,
                            name=f"d_hin{i}")
                 for i in range(len(AR_GROUP_ENDS))]
        d_hout = [dpool.tile([M, (garr[i + 1] - garr[i]) * TC * 3], F16,
                             addr_space="Shared", name=f"d_hout{i}")
                  for i in range(len(AR_GROUP_ENDS))]
        d_g16 = dpool.tile([M, S], F16)

        # resident SBUF (idx in two pieces so the first gather starts early)
        idx_sb = cpool.tile([128, NCH * NIDX // 16], I16)
        nc.sync.dma_start(idx_sb[:, 0:NIDX // 16],
                          t_idx.ap()[:, 0:NIDX // 16])
        nc.sync.dma_start(idx_sb[:, NIDX // 16:],
                          t_idx.ap()[:, NIDX // 16:])
        ftab_sb = cpool.tile([BL, S, 3], F16)
        nc.sync.dma_start(ftab_sb[:], t_ftab.ap())
        sc_sb = cpool.tile([M, 8], F32)
        nc.sync.dma_start(sc_sb[:], t_sc.ap())
        pb_sb = cpool.tile([BL, 1], F32)
        nc.sync.dma_start(pb_sb[:], t_pb.ap())
        ident_sb = cpool.tile([128, 128], F16)
        nc.sync.dma_start(ident_sb[:], t_ident.ap())
        ebblk_sb = cpool.tile([128, S], F16)
        nc.sync.dma_start(ebblk_sb[:], t_ebblk.ap())
        ebts_sb = cpool.tile([M, S, 2], F16)
        nc.sync.dma_start(ebts_sb[:], t_ebts.ap())
        zr = cpool.tile([BL, S], F32)
        eet = cpool.tile([128, S // 2, 128], F16)  # (t%2)*64+m, t//2, b

        hs = cpool.tile([M, S, 3], F16)

        def fire_ar(g):
            """AllReduce stat group g and load the result into hs."""
            lo, hi = garr[g], garr[g + 1]
            if single_core:
                nc.sync.dma_start(d_hout[g][:], d_hin[g][:])
            else:
                nc.gpsimd.collective_compute(
                    "AllReduce", ALU.add,
                    replica_groups=[list(range(NCORES))],
                    ins=[d_hin[g][:].opt()],
                    outs=[d_hout[g][:].opt()],
                )
            nc.sync.dma_start(
                hs[:, lo * TC:hi * TC, :],
                d_hout[g][:].rearrange("m (s k) -> m s k", k=3))

        # ---------------- phase A: per-chunk softmax stats + H matmuls
        actx = ctx.enter_context(ExitStack())
        apool = actx.enter_context(tc.tile_pool(name="pha", bufs=4))
        bpool = actx.enter_context(tc.tile_pool(name="phb", bufs=2))
        epool = actx.enter_context(tc.tile_pool(name="phe", bufs=3))
        spool = actx.enter_context(tc.tile_pool(name="phs", bufs=2))
        pspool = actx.enter_context(
            tc.tile_pool(name="php", bufs=2, space="PSUM"))
        tpool = actx.enter_context(
            tc.tile_pool(name="pht", bufs=2, space="PSUM"))
        zpool = actx.enter_context(
            tc.tile_pool(name="phz", bufs=2, space="PSUM"))
        qctr = 0
        for c in range(NCH):
            ts = slice(c * TC, (c + 1) * TC)
            ge = apool.tile([BL, TC * M], F32, tag="ge")
            ge3 = ge[:].rearrange("p (a b) -> p a b", b=M)
            nsub = NIDX // SPLIT
            tsub = TC // SPLIT
            for a in range(SPLIT):
                i0 = c * NIDX // 16 + a * nsub // 16
                nc.gpsimd.dma_gather(
                    out_ap=ge3[:, a * tsub:(a + 1) * tsub, :],
                    in_ap=t_simtab.ap(),
                    idxs_ap=idx_sb[:, i0:i0 + nsub // 16],
                    num_idxs=nsub,
                    num_idxs_reg=nsub,
                    elem_size=M,
                    single_packet=False,
                    queue_num=qctr % NQUEUES,
                )
                qctr += 1
            # exp(sim) straight to fp16; the per-(t,m) bias factor eb is NOT
            # multiplied in here — it commutes out of every b-contraction, so
            # it is applied to the tiny [M,S]-sized stats (ebts), the softmax
            # denominator (via the ebblk matmul columns) and g (in phase C)
            ge16 = epool.tile([BL, TC * M], F16, tag="ge16")
            nc.scalar.activation(ge16[:], ge[:], ACTF.Exp)
            ee3 = ge16[:].rearrange("p (a b) -> p a b", b=M)
            # transpose into the resident read-dot operand via the PE
            # (dma_start_transpose starves the gather SDMA queues), then
            # drain PSUM->SBUF on the scalar engine to keep the DVE light
            pst = tpool.tile([128, TC // 2, 128], F16, tag="pst")
            eeb = ge16[:].rearrange("p (k b) -> p k b", b=128)
            for kk in range(TC // 2):
                nc.tensor.transpose(pst[:, kk, :], eeb[:, kk, :], ident_sb[:])
            nc.scalar.activation(
                eet[:, c * TC // 2:(c + 1) * TC // 2, :], pst[:], ACTF.Copy)
            # softmax denominator Z[b,t] = sum_m ge*eb via PE pair-matmuls
            # against the block-diagonal eb columns
            psZ = zpool.tile([BL, TC], F32, tag="psZ")
            for jj in range(TC // 2):
                j = c * TC // 2 + jj
                nc.tensor.matmul(psZ[:, 2 * jj:2 * jj + 2],
                                 lhsT=eet[:, j, :],
                                 rhs=ebblk_sb[:, 2 * j:2 * j + 2],
                                 start=True, stop=True)
            zrc = zr[:, ts]
            nc.vector.reciprocal(zrc, psZ[:])
            zr2 = spool.tile([BL, TC], F32, tag="zr2")
            nc.vector.tensor_tensor(zr2[:], zrc, zrc, ALU.mult)
            fp = spool.tile([BL, TC, 3], F16, tag="fp")
            nc.vector.tensor_tensor(
                fp[:, :, 0:2], ftab_sb[:, ts, 0:2],
                zrc.unsqueeze(2).broadcast_to([BL, TC, 2]), ALU.mult)
            nc.vector.tensor_tensor(
                fp[:, :, 2:3], ftab_sb[:, ts, 2:3],
                zr2[:].unsqueeze(2).broadcast_to([BL, TC, 1]), ALU.mult)
            e2 = bpool.tile([BL, TC * M], F16, tag="e2")
            nc.scalar.activation(e2[:], ge16[:], ACTF.Square)
            e23 = e2[:].rearrange("p (a b) -> p a b", b=M)
            hp = pspool.tile([M, TC * 3], F32, tag="hp")
            for t in range(TC):
                nc.tensor.matmul(hp[:, t * 3:t * 3 + 2], lhsT=ee3[:, t, :],
                                 rhs=fp[:, t, 0:2], start=True, stop=True)
                nc.tensor.matmul(hp[:, t * 3 + 2:t * 3 + 3],
                                 lhsT=e23[:, t, :], rhs=fp[:, t, 2:3],
                                 start=True, stop=True)
            # apply the commuted eb / eb^2 factors while draining PSUM
            hbc = spool.tile([M, TC * 3], F16, tag="hbc")
            hb3 = hbc[:].rearrange("m (s k) -> m s k", k=3)
            hp3 = hp[:].rearrange("m (s k) -> m s k", k=3)
            nc.vector.tensor_tensor(
                hb3[:, :, 0:2], hp3[:, :, 0:2],
                ebts_sb[:, ts, 0:1].broadcast_to([M, TC, 2]), ALU.mult)
            nc.vector.tensor_tensor(
                hb3[:, :, 2:3], hp3[:, :, 2:3],
                ebts_sb[:, ts, 1:2].broadcast_to([M, TC, 1]), ALU.mult)
            gidx = next(i for i in range(len(AR_GROUP_ENDS))
                        if c < garr[i + 1])
            off = (c - garr[gidx]) * TC * 3
            nc.scalar.dma_start(d_hin[gidx][:, off:off + TC * 3], hbc[:])
            # fire each group's AllReduce 3 chunks after its last spill, so
            # the trigger's sem wait never blocks the gather descriptor
            # stream on the gpsimd queue
            for g in range(len(AR_GROUP_ENDS)):
                if c == garr[g + 1] - 1 + 3:
                    fire_ar(g)
        for g in range(len(AR_GROUP_ENDS)):
            if garr[g + 1] - 1 + 3 > NCH - 1:
                fire_ar(g)
        actx.close()

        TA = ARSPLIT * TC            # phase C/D split point
        NGA = TA // 128              # phase-D groups fully inside part A
        NG = S // 128

        # ---------------- phase C: cumsums + read vector g  (all [M, S]),
        # split at TA so the part-A compute overlaps AllReduce B.  carry[:, k]
        # holds each scan's part-A total for the part-B fix-up.
        cpool2 = ctx.enter_context(tc.tile_pool(name="phc", bufs=1))
        css = cpool2.tile([M, S], F32)
        rcs = cpool2.tile([M, S], F32)
        cc = cpool2.tile([M, S], F32)
        sfac = cpool2.tile([M, S], F32)
        num = cpool2.tile([M, S], F32)
        den = cpool2.tile([M, S], F32)
        g16 = cpool2.tile([M, S], F16)
        carry = cpool2.tile([M, 8], F32)
        gdup = cpool.tile([128, S], F16)
        gblk = cpool.tile([128, S], F16)
        nc.vector.memset(gblk[:], 0.0)

        def phase_c(lo, hi, first, last):
            sl = slice(lo, hi)
            n = hi - lo
            tag = str(lo)
            nc.vector.tensor_scalar_add(css[:, sl], hs[:, sl, 0], 1e-8)
            nc.vector.reciprocal(rcs[:, sl], css[:, sl])
            nc.vector.tensor_tensor(cc[:, sl], hs[:, sl, 1], hs[:, sl, 0],
                                    ALU.mult)
            nc.vector.tensor_tensor(cc[:, sl], cc[:, sl], rcs[:, sl],
                                    ALU.mult)
            nc.vector.tensor_scalar(cc[:, sl], cc[:, sl], 1.0 / B, None,
                                    ALU.mult)
            nc.vector.tensor_tensor(sfac[:, sl], cc[:, sl], rcs[:, sl],
                                    ALU.mult)
            for k in range(2):
                # k == 0: the gp-weighted evidence cumsum (num);
                # k == 1: the precision cumsum over cc (den)
                src = cc[:, sl] if k == 1 else None
                ch = cpool2.tile([M, n + 1], F32, tag="ch" + tag)
                if k == 0:
                    hsk = cpool2.tile([M, n], F32, tag="hsk" + tag)
                    nc.vector.tensor_tensor(hsk[:], hs[:, sl, 2],
                                            sfac[:, sl], ALU.mult)
                    src = hsk[:]
                nc.vector.memset(ch[:, 0:1], 0.0)
                nc.vector.tensor_tensor_scan(ch[:, 1:n + 1], src, src, 0.0,
                                             ALU.add, ALU.bypass)
                if not first:
                    # shift the whole local scan by the running carry; this
                    # also turns ch[n] into the new global inclusive total
                    nc.vector.tensor_scalar_add(ch[:, 0:n + 1], ch[:, 0:n + 1],
                                                carry[:, k:k + 1])
                if not last:
                    nc.vector.tensor_copy(carry[:, k:k + 1], ch[:, n:n + 1])
                if k == 0:
                    nc.vector.tensor_scalar_add(num[:, sl], ch[:, 0:n],
                                                sc_sb[:, 5:6])
                else:
                    nc.vector.tensor_scalar_add(den[:, sl], ch[:, 0:n],
                                                sc_sb[:, 4:5])
            # g = num / den, times the commuted eb factor, scaled into the
            # fp16 normal range
            nc.vector.reciprocal(den[:, sl], den[:, sl])
            nc.vector.tensor_tensor(num[:, sl], num[:, sl], den[:, sl],
                                    ALU.mult)
            nc.vector.tensor_tensor(num[:, sl], num[:, sl],
                                    ebts_sb[:, sl, 0], ALU.mult)
            nc.vector.tensor_scalar(g16[:, sl], num[:, sl], GSCALE, None,
                                    ALU.mult)
            # build the block-diagonal column pairs: col 2j keeps only the
            # even-t (top) half, col 2j+1 only the odd-t (bottom) half; the
            # bottom half needs g16 shifted onto partitions 64-127 (direct
            # SBUF->SBUF partition-offset DMA)
            nc.sync.dma_start(gdup[M:128, sl], g16[:, sl])
            gs2 = g16[:, sl].rearrange("p (j two) -> p j two", two=2)
            gd2 = gdup[:, sl].rearrange("p (j two) -> p j two", two=2)
            gb2 = gblk[:, sl].rearrange("p (j two) -> p j two", two=2)
            nc.vector.tensor_copy(gb2[0:M, :, 0], gs2[:, :, 0])
            nc.vector.tensor_copy(gb2[M:128, :, 1], gd2[M:128, :, 1])

        # ---------------- phase D: dot[b, 2j:2j+2] = eet_pair.T @ gblk_pair
        # (the block-diagonal gblk columns keep even/odd t separate), landing
        # directly in [b, t] layout; preds = zr/GSCALE * dot + pred_b
        rtile = cpool.tile([BL, S], F32)
        dpool2 = ctx.enter_context(tc.tile_pool(name="phd", bufs=2))
        psd = ctx.enter_context(
            tc.tile_pool(name="phdp", bufs=4, space="PSUM"))
        wpool = ctx.enter_context(
            tc.tile_pool(name="phw", bufs=1, space="PSUM"))
        psDs = {}

        def phase_d_mm(jlo, jhi):
            for j in range(jlo, jhi):
                gi = j // 64
                if gi not in psDs:
                    psDs[gi] = psd.tile([BL, 128], F32, tag="psD",
                                        name=f"psD{gi}")
                jj = j % 64
                nc.tensor.matmul(psDs[gi][:, 2 * jj:2 * jj + 2],
                                 lhsT=eet[:, j, :],
                                 rhs=gblk[:, 2 * j:2 * j + 2],
                                 start=True, stop=True)

        def phase_d_fin(glo, ghi):
            for gi in range(glo, ghi):
                gsl = slice(gi * 128, (gi + 1) * 128)
                rt32 = dpool2.tile([BL, 128], F32, tag="rt32")
                nc.vector.tensor_tensor(rt32[:], psDs[gi][:], zr[:, gsl],
                                        ALU.mult)
                nc.vector.tensor_scalar(rtile[:, gsl], rt32[:], 1.0 / GSCALE,
                                        pb_sb[:, 0:1], ALU.mult, ALU.add)

        # three-way tail: t<384 after the first three collectives, 384-448
        # after group 4 (chunks 12-13), the last 64 t after groups 5-6
        phase_c(0, TA, True, False)
        phase_d_mm(0, TA // 2)
        phase_c(TA, 448, False, False)
        phase_d_mm(TA // 2, 224)
        # keep the PE's HAM clock warm across the gap while the last stat
        # collectives land; anchored on the last eet chunk so these fill
        # the gap rather than running early
        warm = wpool.tile([128, 128], F16)
        for _ in range(48):
            nc.tensor.transpose(warm[:], eet[:, S // 2 - 1, :], ident_sb[:])
        phase_c(448, S, False, True)
        phase_d_mm(224, S // 2)
        phase_d_fin(0, NG)
        nc.sync.dma_start(t_preds.ap(), rtile[:])


def _softplus(x):
    return np.logaddexp(0.0, x)


def _host_prep(inputs):
    """All the cheap host-side precomputation; returns per-core in_maps."""
    q_ids = np.asarray(inputs["q_ids"], np.int64)          # [B, S]
    responses = np.asarray(inputs["responses"], np.int64)  # [B, S]
    q_table = np.asarray(inputs["q_table"], np.float32)
    key_embeds = np.asarray(inputs["key_embeds"], np.float32)
    alpha_mean = np.asarray(inputs["alpha_mean"], np.float32)
    alpha_log_var = np.asarray(inputs["alpha_log_var"], np.float32)
    beta_base = np.asarray(inputs["beta_base"], np.float32)
    beta_offsets = np.asarray(inputs["beta_offsets"], np.float32)
    theta_mean0 = np.asarray(inputs["theta_mean0"], np.float32)
    theta_log_var0 = np.asarray(inputs["theta_log_var0"], np.float32)
    q2k_w = np.asarray(inputs["q2k_w"], np.float32)
    q2k_b = np.asarray(inputs["q2k_b"], np.float32)
    qa_w = np.asarray(inputs["qa_w"], np.float32)
    qa_b = np.asarray(inputs["qa_b"], np.float32)
    qae_w = np.asarray(inputs["qae_w"], np.float32)
    qae_b = np.asarray(inputs["qae_b"], np.float32)
    pred_w = np.asarray(inputs["pred_w"], np.float32)
    pred_b = np.asarray(inputs["pred_b"], np.float32)
    alpha_noise = np.asarray(inputs["alpha_noise"], np.float32)
    beta_noise = np.asarray(inputs["beta_noise"], np.float32)

    # sim table: folds q_table @ q2k_w @ key_embeds.T (+ bias) into a gather
    w_qm = q2k_w @ key_embeds.T                            # [E, M]
    b_m = q2k_b @ key_embeds.T                             # [M]
    simtab = (q_table @ w_qm + b_m[None]).astype(np.float32)

    # per-(t, m) logit bias -> eb = exp(bias)
    alpha = np.exp(alpha_mean[None] + alpha_noise
                   * np.exp(0.5 * alpha_log_var)[None])    # [S, M]
    base = beta_base[None] + beta_noise * 0.1              # [S, M]
    offs = _softplus(beta_offsets)                         # [M, C-1]
    cum = np.concatenate([np.zeros((M, 1), np.float32),
                          np.cumsum(offs, 1)[:, :C - 2]], 1)
    beta_mean = base + cum.mean(1)[None]
    diff_sim = np.exp(-0.5 * beta_mean ** 2)
    ebt = np.exp(0.3 * alpha + 0.2 * diff_sim).astype(np.float32)  # [S, M]
    # block-diagonal eb columns for the on-device Z matmuls: col 2j keeps
    # the even-t value on the top partition half, col 2j+1 the odd-t value
    # on the bottom half
    ebblk = np.zeros((128, S), np.float16)
    ebblk[0:M, 0::2] = ebt.T[:, 0::2]
    ebblk[M:128, 1::2] = ebt.T[:, 1::2]
    # eb and eb^2 per (m, t) for the commuted H-stat scaling
    ebts = np.empty((M, S, 2), np.float16)
    ebts[:, :, 0] = ebt.T
    ebts[:, :, 1] = (ebt.T ** 2)

    # evidence scalars per (b, t)
    rn = responses.astype(np.float32) / (C - 1)
    p = np.clip(rn, 0.01, 0.99)
    ae = np.log(p) - np.log1p(-p)
    pr = 0.5 + np.abs(rn - 0.5) * 2.0
    q01 = q_ids.astype(np.float32) / NQ

    # rank-4 decomposition of comb over V
    w0v = qa_w[0] @ qae_w
    w1v = qa_w[1] @ qae_w
    bv = qa_b @ qae_w + qae_b
    pw = pred_w[:, 0]
    gp = 0.5 * np.array([w0v @ pw, w1v @ pw, bv @ pw, pw.sum()], np.float32)

    alo = np.exp(-theta_log_var0[:, 0])                    # [M]
    n0pw = alo * (theta_mean0 @ pw)                        # [M]
    sc = np.zeros((M, 8), np.float32)
    sc[:, 0:4] = gp[None, :]
    sc[:, 4] = alo
    sc[:, 5] = n0pw

    pb = np.full((BL, 1), float(pred_b[0]), np.float32)
    ident = np.eye(128, dtype=np.float16)

    in_maps = []
    for core in range(NCORES):
        bs = slice(core * BL, (core + 1) * BL)
        qs = q_ids[bs]                                     # [128, S]
        # gather indices, chunk-major, wrapped in 16 partitions
        blocks = []
        for c in range(NCH):
            flat = qs[:, c * TC:(c + 1) * TC].T.reshape(-1)  # t-major
            w16 = flat.reshape(NIDX // 16, 16).T             # [16, NIDX/16]
            blocks.append(np.tile(w16, (8, 1)))
        idx16 = np.concatenate(blocks, axis=1).astype(np.int16)

        # the four evidence features only ever enter through the fixed
        # combination sum_k gp_k * f_k (gp commutes through the cumsum),
        # so fold them into a single column on the host
        ftab = np.empty((BL, S, 3), np.float16)
        ftab[:, :, 0] = 1.0
        ftab[:, :, 1] = pr[bs]
        ftab[:, :, 2] = (gp[0] * q01[bs] + gp[1] * rn[bs] + gp[2]
                         + gp[3] * ae[bs])

        in_maps.append({
            "simtab": simtab,
            "idx16": idx16,
            "ftab": ftab,
            "ebblk": ebblk,
            "ebts": ebts,
            "scal": sc,
            "pb": pb,
            "ident": ident,
        })
    return in_maps


def _run(in_maps, **kw):
    if "nc" not in _CACHE:
        _CACHE["nc"] = _build()
    res = run_bass_kernel_spmd(_CACHE["nc"], in_maps,
                               core_ids=list(range(NCORES)), **kw)
    preds = np.concatenate([res.results[c]["preds"] for c in range(NCORES)],
                           axis=0)
    return preds.astype(np.float32), res


def kernel(**inputs) -> np.ndarray:
    return _run(_host_prep(inputs))[0]


if __name__ == "__main__":
    pass


# revision 62
# speedup vs baseline: 1.1612x; 1.1612x over previous
"""Trainium2 Bass kernel for DeepBayesianDKVMN (nn_DeepBayesianDKVMN_39857296507058).

Math restructuring
------------------
The reference's sequential Bayesian-write scan is *linear* in the memory
state: the per-step precision/evidence increments depend only on step-t
inputs, never on the evolving state.  So the scan collapses to exclusive
cumulative sums over time, and everything else is batch-parallel:

  - front end: q_table[q_ids] @ q2k_w @ key_embeds.T folds into ONE
    [NQ+1, M] "sim table" gather (host precomputes the table, device does
    a dma_gather of 256B rows).
  - softmax: logits = sim + bias[t,m]; exp(bias) is multiplied in as eb.
  - per-(b,t) evidence vector comb[b,t,:] is a rank-4 combination of four
    fixed V-vectors, and those four only ever enter through one fixed
    gp-weighted combination, so the [S,M,V] write-aggregation reduces to
    THREE [S,M] batch sums: colsum, P (precision), Hw (weighted evidence).
  - the 1/Z softmax normalizations fold into the per-(b,t) feature matrix
    F' = F * [1/Z, 1/Z, 1/Z^2], contracted against [ee | ee^2] by tiny
    per-t PE matmuls (contraction over the 128 batch rows on partitions).
  - the per-(t,m) logit-bias factor eb commutes out of every
    b-contraction, so it is never materialized per-(b,t,m): it scales the
    [M,S] stats (ebts), forms the Z matmul columns (ebblk), and folds
    into the read vector g.
  - AllReduce over the three [S,M] stats; exclusive cumsums via
    tensor_tensor_scan; read vector g[t,m]; preds = zr * (ee . g) + pred_b.

Performance structure
---------------------
  - the dma_gather descriptor stream is spread over all 4 SWDGE queues
    (4 sub-gathers x 16 chunks, round-robin) so four SDMA engines drain
    concurrently; single-queue drain was the original 512us bottleneck.
  - 16 chunks of 32 timesteps pipeline gather/exp/stats; nothing else is
    allowed to produce bulk SDMA traffic during the gather (HWDGE
    transposes / broadcast streams starve the SWDGE drains), so chunk
    transposes run on the PE (via identity matmuls) and PSUM drains on
    the scalar engine.
  - exp(sim) chunks are PE-transposed into a resident [2t x 64m, b]
    tensor; Z and the final read-dot are pair-matmuls against
    block-diagonal [128, 2] column slices (ebblk / gblk), landing in
    [b, t] layout with no output transpose.
  - the stat AllReduce is slice-rate bound on the CC stream
    (~0.69us/2048 elements + ~8us/op rendezvous), so it is split into 6
    groups fired as their chunks complete; only the last 1-chunk group is
    exposed past the gather.
  - phase C (cumsums -> g) and phase D (read-dot) are each split at
    t=384 with per-scan carries, overlapping the collective tail; dummy
    PE transposes bridge the idle gap so the HAM clock stays at 2.4GHz
    for the phase-D weight loads.

Sharding: batch 1024 -> 128 rows per core across 8 cores (data parallel),
as the sharding hint suggests; the all-reduce is the per-slot aggregated
evidence/precision, shrunk by the rank-4 trick.
"""

import numpy as np
from contextlib import ExitStack

import concourse.bass as bass
import concourse.tile as tile
from concourse import bacc, mybir
from concourse.bass_utils import run_bass_kernel_spmd

# problem dims (hardcoded per spec)
B, S, M, K, V, E, NQ, C = 1024, 512, 64, 64, 128, 64, 10000, 4
NCORES = 8
BL = B // NCORES            # 128 batch rows per core
TC = 32                     # timestep chunk
NCH = S // TC               # 16 chunks
NIDX = BL * TC              # gather indices per chunk = 4096
SPLIT = 4                   # sub-gathers per chunk (queue spreading)
NQUEUES = 4                 # SWDGE queues (max 4)
ARSPLIT = 12                # chunks in phase C/D's first (overlapped) part
AR_GROUP_ENDS = (4, 8, 12, 14, 15, 16)  # stat-collective group boundaries
F32 = mybir.dt.float32
F16 = mybir.dt.float16
I16 = mybir.dt.int16
GSCALE = 1024.0            # fp16 pre-scale for the tiny g values
ALU = mybir.AluOpType
AXT = mybir.AxisListType
ACTF = mybir.ActivationFunctionType

_CACHE = {}


def _build(single_core=False):
    nc = bacc.Bacc("TRN2", target_bir_lowering=False, debug=False,
                   num_devices=1 if single_core else NCORES,
                   num_swdge_queues=NQUEUES)

    t_simtab = nc.dram_tensor("simtab", [NQ + 1, M], F32, kind="ExternalInput")
    t_idx = nc.dram_tensor("idx16", [128, NCH * NIDX // 16], I16,
                           kind="ExternalInput")
    t_ftab = nc.dram_tensor("ftab", [BL, S, 3], F16, kind="ExternalInput")
    t_ebblk = nc.dram_tensor("ebblk", [128, S], F16, kind="ExternalInput")
    t_ebts = nc.dram_tensor("ebts", [M, S, 2], F16, kind="ExternalInput")
    t_sc = nc.dram_tensor("scal", [M, 8], F32, kind="ExternalInput")
    t_pb = nc.dram_tensor("pb", [BL, 1], F32, kind="ExternalInput")
    t_ident = nc.dram_tensor("ident", [128, 128], F16, kind="ExternalInput")
    t_preds = nc.dram_tensor("preds", [BL, S], F32, kind="ExternalOutput")

    with tile.TileContext(nc) as tc:
        _build_body(nc, tc, single_core, t_simtab, t_idx, t_ftab, t_ebblk,
                    t_ebts, t_sc, t_pb, t_ident, t_preds)
    nc.compile()
    return nc


def _build_body(nc, tc, single_core, t_simtab, t_idx, t_ftab, t_ebblk,
                t_ebts, t_sc, t_pb, t_ident, t_preds):
    with ExitStack() as ctx:
        cpool = ctx.enter_context(tc.tile_pool(name="const", bufs=1))
        dpool = ctx.enter_context(tc.tile_pool(name="dram", bufs=1,
                                               space="DRAM"))
        # internal DRAM: grouped stat buffers.  The CC stream costs
        # ~4.3us/op + 0.69us per 2048-element slice, so the AllReduce is
        # split into a few groups fired as their chunks complete; only the
        # last (small) group is exposed after the gather phase.
        garr = [0] + list(AR_GROUP_ENDS)
        d_hin = [dpool.tile([M, (garr[i + 1] - garr[i]) * TC * 3], F16,
                            name=f"d_hin{i}")
                 for i in range(len(AR_GROUP_ENDS))]
        d_hout = [dpool.tile([M, (garr[i + 1] - garr[i]) * TC * 3], F16,
                             addr_space="Shared", name=f"d_hout{i}")
                  for i in range(len(AR_GROUP_ENDS))]
        d_g16 = dpool.tile([M, S], F16)

        # resident SBUF (idx in two pieces so the first gather starts early)
        idx_sb = cpool.tile([128, NCH * NIDX // 16], I16)
        nc.sync.dma_start(idx_sb[:, 0:NIDX // 16],
                          t_idx.ap()[:, 0:NIDX // 16])
        nc.sync.dma_start(idx_sb[:, NIDX // 16:],
                          t_idx.ap()[:, NIDX // 16:])
        ftab_sb = cpool.tile([BL, S, 3], F16)
        nc.sync.dma_start(ftab_sb[:], t_ftab.ap())
        sc_sb = cpool.tile([M, 8], F32)
        nc.sync.dma_start(sc_sb[:], t_sc.ap())
        pb_sb = cpool.tile([BL, 1], F32)
        nc.sync.dma_start(pb_sb[:], t_pb.ap())
        ident_sb = cpool.tile([128, 128], F16)
        nc.sync.dma_start(ident_sb[:], t_ident.ap())
        ebblk_sb = cpool.tile([128, S], F16)
        nc.sync.dma_start(ebblk_sb[:], t_ebblk.ap())
        ebts_sb = cpool.tile([M, S, 2], F16)
        nc.sync.dma_start(ebts_sb[:], t_ebts.ap())
        zr = cpool.tile([BL, S], F32)
        eet = cpool.tile([128, S // 2, 128], F16)  # (t%2)*64+m, t//2, b

        hs = cpool.tile([M, S, 3], F16)

        def fire_ar(g):
            """AllReduce stat group g and load the result into hs."""
            lo, hi = garr[g], garr[g + 1]
            if single_core:
                nc.sync.dma_start(d_hout[g][:], d_hin[g][:])
            else:
                nc.gpsimd.collective_compute(
                    "AllReduce", ALU.add,
                    replica_groups=[list(range(NCORES))],
                    ins=[d_hin[g][:].opt()],
                    outs=[d_hout[g][:].opt()],
                )
            nc.sync.dma_start(
                hs[:, lo * TC:hi * TC, :],
                d_hout[g][:].rearrange("m (s k) -> m s k", k=3))

        # ---------------- phase A: per-chunk softmax stats + H matmuls
        actx = ctx.enter_context(ExitStack())
        apool = actx.enter_context(tc.tile_pool(name="pha", bufs=4))
        bpool = actx.enter_context(tc.tile_pool(name="phb", bufs=2))
        epool = actx.enter_context(tc.tile_pool(name="phe", bufs=3))
        spool = actx.enter_context(tc.tile_pool(name="phs", bufs=2))
        pspool = actx.enter_context(
            tc.tile_pool(name="php", bufs=2, space="PSUM"))
        tpool = actx.enter_context(
            tc.tile_pool(name="pht", bufs=2, space="PSUM"))
        zpool = actx.enter_context(
            tc.tile_pool(name="phz", bufs=2, space="PSUM"))
        qctr = 0
        for c in range(NCH):
            ts = slice(c * TC, (c + 1) * TC)
            ge = apool.tile([BL, TC * M], F32, tag="ge")
            ge3 = ge[:].rearrange("p (a b) -> p a b", b=M)
            nsub = NIDX // SPLIT
            tsub = TC // SPLIT
            for a in range(SPLIT):
                i0 = c * NIDX // 16 + a * nsub // 16
                nc.gpsimd.dma_gather(
                    out_ap=ge3[:, a * tsub:(a + 1) * tsub, :],
                    in_ap=t_simtab.ap(),
                    idxs_ap=idx_sb[:, i0:i0 + nsub // 16],
                    num_idxs=nsub,
                    num_idxs_reg=nsub,
                    elem_size=M,
                    single_packet=False,
                    queue_num=qctr % NQUEUES,
                )
                qctr += 1
            # exp(sim) straight to fp16; the per-(t,m) bias factor eb is NOT
            # multiplied in here — it commutes out of every b-contraction, so
            # it is applied to the tiny [M,S]-sized stats (ebts), the softmax
            # denominator (via the ebblk matmul columns) and g (in phase C)
            ge16 = epool.tile([BL, TC * M], F16, tag="ge16")
            nc.scalar.activation(ge16[:], ge[:], ACTF.Exp)
            ee3 = ge16[:].rearrange("p (a b) -> p a b", b=M)
            # transpose into the resident read-dot operand via the PE
            # (dma_start_transpose starves the gather SDMA queues), then
            # drain PSUM->SBUF on the scalar engine to keep the DVE light
            pst = tpool.tile([128, TC // 2, 128], F16, tag="pst")
            eeb = ge16[:].rearrange("p (k b) -> p k b", b=128)
            for kk in range(TC // 2):
                nc.tensor.transpose(pst[:, kk, :], eeb[:, kk, :], ident_sb[:])
            nc.scalar.activation(
                eet[:, c * TC // 2:(c + 1) * TC // 2, :], pst[:], ACTF.Copy)
            # softmax denominator Z[b,t] = sum_m ge*eb via PE pair-matmuls
            # against the block-diagonal eb columns
            psZ = zpool.tile([BL, TC], F32, tag="psZ")
            for jj in range(TC // 2):
                j = c * TC // 2 + jj
                nc.tensor.matmul(psZ[:, 2 * jj:2 * jj + 2],
                                 lhsT=eet[:, j, :],
                                 rhs=ebblk_sb[:, 2 * j:2 * j + 2],
                                 start=True, stop=True)
            zrc = zr[:, ts]
            nc.vector.reciprocal(zrc, psZ[:])
            zr2 = spool.tile([BL, TC], F32, tag="zr2")
            nc.vector.tensor_tensor(zr2[:], zrc, zrc, ALU.mult)
            fp = spool.tile([BL, TC, 3], F16, tag="fp")
            nc.vector.tensor_tensor(
                fp[:, :, 0:2], ftab_sb[:, ts, 0:2],
                zrc.unsqueeze(2).broadcast_to([BL, TC, 2]), ALU.mult)
            nc.vector.tensor_tensor(
                fp[:, :, 2:3], ftab_sb[:, ts, 2:3],
                zr2[:].unsqueeze(2).broadcast_to([BL, TC, 1]), ALU.mult)
            e2 = bpool.tile([BL, TC * M], F16, tag="e2")
            nc.scalar.activation(e2[:], ge16[:], ACTF.Square)
            e23 = e2[:].rearrange("p (a b) -> p a b", b=M)
            hp = pspool.tile([M, TC * 3], F32, tag="hp")
            for t in range(TC):
                nc.tensor.matmul(hp[:, t * 3:t * 3 + 2], lhsT=ee3[:, t, :],
                                 rhs=fp[:, t, 0:2], start=True, stop=True)
                nc.tensor.matmul(hp[:, t * 3 + 2:t * 3 + 3],
                                 lhsT=e23[:, t, :], rhs=fp[:, t, 2:3],
                                 start=True, stop=True)
            # apply the commuted eb / eb^2 factors while draining PSUM
            hbc = spool.tile([M, TC * 3], F16, tag="hbc")
            hb3 = hbc[:].rearrange("m (s k) -> m s k", k=3)
            hp3 = hp[:].rearrange("m (s k) -> m s k", k=3)
            nc.vector.tensor_tensor(
                hb3[:, :, 0:2], hp3[:, :, 0:2],
                ebts_sb[:, ts, 0:1].broadcast_to([M, TC, 2]), ALU.mult)
            nc.vector.tensor_tensor(
                hb3[:, :, 2:3], hp3[:, :, 2:3],
                ebts_sb[:, ts, 1:2].broadcast_to([M, TC, 1]), ALU.mult)
            gidx = next(i for i in range(len(AR_GROUP_ENDS))
                        if c < garr[i + 1])
            off = (c - garr[gidx]) * TC * 3
            nc.scalar.dma_start(d_hin[gidx][:, off:off + TC * 3], hbc[:])
            # fire each group's AllReduce 3 chunks after its last spill, so
            # the trigger's sem wait never blocks the gather descriptor
            # stream on the gpsimd queue
            for g in range(len(AR_GROUP_ENDS)):
                if c == garr[g + 1] - 1 + 3:
                    fire_ar(g)
        for g in range(len(AR_GROUP_ENDS)):
            if garr[g + 1] - 1 + 3 > NCH - 1:
                fire_ar(g)
        actx.close()

        TA = ARSPLIT * TC            # phase C/D split point
        NGA = TA // 128              # phase-D groups fully inside part A
        NG = S // 128

        # ---------------- phase C: cumsums + read vector g  (all [M, S]),
        # split at TA so the part-A compute overlaps AllReduce B.  carry[:, k]
        # holds each scan's part-A total for the part-B fix-up.
        cpool2 = ctx.enter_context(tc.tile_pool(name="phc", bufs=1))
        css = cpool2.tile([M, S], F32)
        rcs = cpool2.tile([M, S], F32)
        cc = cpool2.tile([M, S], F32)
        sfac = cpool2.tile([M, S], F32)
        num = cpool2.tile([M, S], F32)
        den = cpool2.tile([M, S], F32)
        g16 = cpool2.tile([M, S], F16)
        carry = cpool2.tile([M, 8], F32)
        gdup = cpool.tile([128, S], F16)
        gblk = cpool.tile([128, S], F16)
        nc.vector.memset(gblk[:], 0.0)

        def phase_c(lo, hi, first, last):
            sl = slice(lo, hi)
            n = hi - lo
            tag = str(lo)
            nc.vector.tensor_scalar_add(css[:, sl], hs[:, sl, 0], 1e-8)
            nc.vector.reciprocal(rcs[:, sl], css[:, sl])
            nc.vector.tensor_tensor(cc[:, sl], hs[:, sl, 1], hs[:, sl, 0],
                                    ALU.mult)
            nc.vector.tensor_tensor(cc[:, sl], cc[:, sl], rcs[:, sl],
                                    ALU.mult)
            nc.vector.tensor_scalar(cc[:, sl], cc[:, sl], 1.0 / B, None,
                                    ALU.mult)
            nc.vector.tensor_tensor(sfac[:, sl], cc[:, sl], rcs[:, sl],
                                    ALU.mult)
            for k in range(2):
                # k == 0: the gp-weighted evidence cumsum (num);
                # k == 1: the precision cumsum over cc (den)
                src = cc[:, sl] if k == 1 else None
                ch = cpool2.tile([M, n + 1], F32, tag="ch" + tag)
                if k == 0:
                    hsk = cpool2.tile([M, n], F32, tag="hsk" + tag)
                    nc.vector.tensor_tensor(hsk[:], hs[:, sl, 2],
                                            sfac[:, sl], ALU.mult)
                    src = hsk[:]
                nc.vector.memset(ch[:, 0:1], 0.0)
                nc.vector.tensor_tensor_scan(ch[:, 1:n + 1], src, src, 0.0,
                                             ALU.add, ALU.bypass)
                if not first:
                    # shift the whole local scan by the running carry; this
                    # also turns ch[n] into the new global inclusive total
                    nc.vector.tensor_scalar_add(ch[:, 0:n + 1], ch[:, 0:n + 1],
                                                carry[:, k:k + 1])
                if not last:
                    nc.vector.tensor_copy(carry[:, k:k + 1], ch[:, n:n + 1])
                if k == 0:
                    nc.vector.tensor_scalar_add(num[:, sl], ch[:, 0:n],
                                                sc_sb[:, 5:6])
                else:
                    nc.vector.tensor_scalar_add(den[:, sl], ch[:, 0:n],
                                                sc_sb[:, 4:5])
            # g = num / den, times the commuted eb factor, scaled into the
            # fp16 normal range
            nc.vector.reciprocal(den[:, sl], den[:, sl])
            nc.vector.tensor_tensor(num[:, sl], num[:, sl], den[:, sl],
                                    ALU.mult)
            nc.vector.tensor_tensor(num[:, sl], num[:, sl],
                                    ebts_sb[:, sl, 0], ALU.mult)
            nc.vector.tensor_scalar(g16[:, sl], num[:, sl], GSCALE, None,
                                    ALU.mult)
            # bounce via DRAM onto both partition halves, then build the
            # block-diagonal column pairs: col 2j keeps only the even-t
            # (top) half, col 2j+1 only the odd-t (bottom) half
            nc.sync.dma_start(d_g16[:, sl], g16[:, sl])
            nc.sync.dma_start(gdup[0:M, sl], d_g16[:, sl])
            nc.sync.dma_start(gdup[M:128, sl], d_g16[:, sl])
            gd2 = gdup[:, sl].rearrange("p (j two) -> p j two", two=2)
            gb2 = gblk[:, sl].rearrange("p (j two) -> p j two", two=2)
            nc.vector.tensor_copy(gb2[0:M, :, 0], gd2[0:M, :, 0])
            nc.vector.tensor_copy(gb2[M:128, :, 1], gd2[M:128, :, 1])

        # ---------------- phase D: dot[b, 2j:2j+2] = eet_pair.T @ gblk_pair
        # (the block-diagonal gblk columns keep even/odd t separate), landing
        # directly in [b, t] layout; preds = zr/GSCALE * dot + pred_b
        rtile = cpool.tile([BL, S], F32)
        dpool2 = ctx.enter_context(tc.tile_pool(name="phd", bufs=2))
        psd = ctx.enter_context(
            tc.tile_pool(name="phdp", bufs=4, space="PSUM"))
        wpool = ctx.enter_context(
            tc.tile_pool(name="phw", bufs=1, space="PSUM"))
        psDs = {}

        def phase_d_mm(jlo, jhi):
            for j in range(jlo, jhi):
                gi = j // 64
                if gi not in psDs:
                    psDs[gi] = psd.tile([BL, 128], F32, tag="psD",
                                        name=f"psD{gi}")
                jj = j % 64
                nc.tensor.matmul(psDs[gi][:, 2 * jj:2 * jj + 2],
                                 lhsT=eet[:, j, :],
                                 rhs=gblk[:, 2 * j:2 * j + 2],
                                 start=True, stop=True)

        def phase_d_fin(glo, ghi):
            for gi in range(glo, ghi):
                gsl = slice(gi * 128, (gi + 1) * 128)
                rt32 = dpool2.tile([BL, 128], F32, tag="rt32")
                nc.vector.tensor_tensor(rt32[:], psDs[gi][:], zr[:, gsl],
                                        ALU.mult)
                nc.vector.tensor_scalar(rtile[:, gsl], rt32[:], 1.0 / GSCALE,
                                        pb_sb[:, 0:1], ALU.mult, ALU.add)

        # three-way tail: t<384 after the first three collectives, 384-448
        # after group 4 (chunks 12-13), the last 64 t after groups 5-6
        phase_c(0, TA, True, False)
        phase_d_mm(0, TA // 2)
        phase_c(TA, 448, False, False)
        phase_d_mm(TA // 2, 224)
        # keep the PE's HAM clock warm across the gap while the last stat
        # collectives land; anchored on the last eet chunk so these fill
        # the gap rather than running early
        warm = wpool.tile([128, 128], F16)
        for _ in range(48):
            nc.tensor.transpose(warm[:], eet[:, S // 2 - 1, :], ident_sb[:])
        phase_c(448, S, False, True)
        phase_d_mm(224, S // 2)
        phase_d_fin(0, NG)
        nc.sync.dma_start(t_preds.ap(), rtile[:])


def _softplus(x):
    return np.logaddexp(0.0, x)


def _host_prep(inputs):
    """All the cheap host-side precomputation; returns per-core in_maps."""
    q_ids = np.asarray(inputs["q_ids"], np.int64)          # [B, S]
    responses = np.asarray(inputs["responses"], np.int64)  # [B, S]
    q_table = np.asarray(inputs["q_table"], np.float32)
    key_embeds = np.asarray(inputs["key_embeds"], np.float32)
    alpha_mean = np.asarray(inputs["alpha_mean"], np.float32)
    alpha_log_var = np.asarray(inputs["alpha_log_var"], np.float32)
    beta_base = np.asarray(inputs["beta_base"], np.float32)
    beta_offsets = np.asarray(inputs["beta_offsets"], np.float32)
    theta_mean0 = np.asarray(inputs["theta_mean0"], np.float32)
    theta_log_var0 = np.asarray(inputs["theta_log_var0"], np.float32)
    q2k_w = np.asarray(inputs["q2k_w"], np.float32)
    q2k_b = np.asarray(inputs["q2k_b"], np.float32)
    qa_w = np.asarray(inputs["qa_w"], np.float32)
    qa_b = np.asarray(inputs["qa_b"], np.float32)
    qae_w = np.asarray(inputs["qae_w"], np.float32)
    qae_b = np.asarray(inputs["qae_b"], np.float32)
    pred_w = np.asarray(inputs["pred_w"], np.float32)
    pred_b = np.asarray(inputs["pred_b"], np.float32)
    alpha_noise = np.asarray(inputs["alpha_noise"], np.float32)
    beta_noise = np.asarray(inputs["beta_noise"], np.float32)

    # sim table: folds q_table @ q2k_w @ key_embeds.T (+ bias) into a gather
    w_qm = q2k_w @ key_embeds.T                            # [E, M]
    b_m = q2k_b @ key_embeds.T                             # [M]
    simtab = (q_table @ w_qm + b_m[None]).astype(np.float32)

    # per-(t, m) logit bias -> eb = exp(bias)
    alpha = np.exp(alpha_mean[None] + alpha_noise
                   * np.exp(0.5 * alpha_log_var)[None])    # [S, M]
    base = beta_base[None] + beta_noise * 0.1              # [S, M]
    offs = _softplus(beta_offsets)                         # [M, C-1]
    cum = np.concatenate([np.zeros((M, 1), np.float32),
                          np.cumsum(offs, 1)[:, :C - 2]], 1)
    beta_mean = base + cum.mean(1)[None]
    diff_sim = np.exp(-0.5 * beta_mean ** 2)
    ebt = np.exp(0.3 * alpha + 0.2 * diff_sim).astype(np.float32)  # [S, M]
    # block-diagonal eb columns for the on-device Z matmuls: col 2j keeps
    # the even-t value on the top partition half, col 2j+1 the odd-t value
    # on the bottom half
    ebblk = np.zeros((128, S), np.float16)
    ebblk[0:M, 0::2] = ebt.T[:, 0::2]
    ebblk[M:128, 1::2] = ebt.T[:, 1::2]
    # eb and eb^2 per (m, t) for the commuted H-stat scaling
    ebts = np.empty((M, S, 2), np.float16)
    ebts[:, :, 0] = ebt.T
    ebts[:, :, 1] = (ebt.T ** 2)

    # evidence scalars per (b, t)
    rn = responses.astype(np.float32) / (C - 1)
    p = np.clip(rn, 0.01, 0.99)
    ae = np.log(p) - np.log1p(-p)
    pr = 0.5 + np.abs(rn - 0.5) * 2.0
    q01 = q_ids.astype(np.float32) / NQ

    # rank-4 decomposition of comb over V
    w0v = qa_w[0] @ qae_w
    w1v = qa_w[1] @ qae_w
    bv = qa_b @ qae_w + qae_b
    pw = pred_w[:, 0]
    gp = 0.5 * np.array([w0v @ pw, w1v @ pw, bv @ pw, pw.sum()], np.float32)

    alo = np.exp(-theta_log_var0[:, 0])                    # [M]
    n0pw = alo * (theta_mean0 @ pw)                        # [M]
    sc = np.zeros((M, 8), np.float32)
    sc[:, 0:4] = gp[None, :]
    sc[:, 4] = alo
    sc[:, 5] = n0pw

    pb = np.full((BL, 1), float(pred_b[0]), np.float32)
    ident = np.eye(128, dtype=np.float16)

    in_maps = []
    for core in range(NCORES):
        bs = slice(core * BL, (core + 1) * BL)
        qs = q_ids[bs]                                     # [128, S]
        # gather indices, chunk-major, wrapped in 16 partitions
        blocks = []
        for c in range(NCH):
            flat = qs[:, c * TC:(c + 1) * TC].T.reshape(-1)  # t-major
            w16 = flat.reshape(NIDX // 16, 16).T             # [16, NIDX/16]
            blocks.append(np.tile(w16, (8, 1)))
        idx16 = np.concatenate(blocks, axis=1).astype(np.int16)

        # the four evidence features only ever enter through the fixed
        # combination sum_k gp_k * f_k (gp commutes through the cumsum),
        # so fold them into a single column on the host
        ftab = np.empty((BL, S, 3), np.float16)
        ftab[:, :, 0] = 1.0
        ftab[:, :, 1] = pr[bs]
        ftab[:, :, 2] = (gp[0] * q01[bs] + gp[1] * rn[bs] + gp[2]
                         + gp[3] * ae[bs])

        in_maps.append({
            "simtab": simtab,
            "idx16": idx16,
            "ftab": ftab,
            "ebblk": ebblk,
            "ebts": ebts,
            "scal": sc,
            "pb": pb,
            "ident": ident,
        })
    return in_maps


def _run(in_maps, **kw):
    if "nc" not in _CACHE:
        _CACHE["nc"] = _build()
    res = run_bass_kernel_spmd(_CACHE["nc"], in_maps,
                               core_ids=list(range(NCORES)), **kw)
    preds = np.concatenate([res.results[c]["preds"] for c in range(NCORES)],
                           axis=0)
    return preds.astype(np.float32), res


def kernel(**inputs) -> np.ndarray:
    return _run(_host_prep(inputs))[0]


if __name__ == "__main__":
    pass


# revision 67
# speedup vs baseline: 1.1914x; 1.0259x over previous
"""Trainium2 Bass kernel for DeepBayesianDKVMN (nn_DeepBayesianDKVMN_39857296507058).

Math restructuring
------------------
The reference's sequential Bayesian-write scan is *linear* in the memory
state: the per-step precision/evidence increments depend only on step-t
inputs, never on the evolving state.  So the scan collapses to exclusive
cumulative sums over time, and everything else is batch-parallel:

  - front end: q_table[q_ids] @ q2k_w @ key_embeds.T folds into ONE
    [NQ+1, M] "sim table" gather (host precomputes the table, device does
    a dma_gather of 256B rows).
  - softmax: logits = sim + bias[t,m]; exp(bias) is multiplied in as eb.
  - per-(b,t) evidence vector comb[b,t,:] is a rank-4 combination of four
    fixed V-vectors, and those four only ever enter through one fixed
    gp-weighted combination, so the [S,M,V] write-aggregation reduces to
    THREE [S,M] batch sums: colsum, P (precision), Hw (weighted evidence).
  - the 1/Z softmax normalizations fold into the per-(b,t) feature matrix
    F' = F * [1/Z, 1/Z, 1/Z^2], contracted against [ee | ee^2] by tiny
    per-t PE matmuls (contraction over the 128 batch rows on partitions).
  - the per-(t,m) logit-bias factor eb commutes out of every
    b-contraction, so it is never materialized per-(b,t,m): it scales the
    [M,S] stats (ebts), forms the Z matmul columns (ebblk), and folds
    into the read vector g.
  - AllReduce over the three [S,M] stats; exclusive cumsums via
    tensor_tensor_scan; read vector g[t,m]; preds = zr * (ee . g) + pred_b.

Performance structure
---------------------
  - the dma_gather descriptor stream is spread over all 4 SWDGE queues
    (4 sub-gathers x 16 chunks, round-robin) so four SDMA engines drain
    concurrently; single-queue drain was the original 512us bottleneck.
  - 16 chunks of 32 timesteps pipeline gather/exp/stats; nothing else is
    allowed to produce bulk SDMA traffic during the gather (HWDGE
    transposes / broadcast streams starve the SWDGE drains), so chunk
    transposes run on the PE (via identity matmuls) and PSUM drains on
    the scalar engine.
  - exp(sim) chunks are PE-transposed into a resident [2t x 64m, b]
    tensor; Z and the final read-dot are pair-matmuls against
    block-diagonal [128, 2] column slices (ebblk / gblk), landing in
    [b, t] layout with no output transpose.
  - the stat AllReduce is slice-rate bound on the CC stream
    (~0.69us/2048 elements + ~8us/op rendezvous), so it is split into 6
    groups fired as their chunks complete; only the last 1-chunk group is
    exposed past the gather.
  - phase C (cumsums -> g) and phase D (read-dot) are each split at
    t=384 with per-scan carries, overlapping the collective tail; dummy
    PE transposes bridge the idle gap so the HAM clock stays at 2.4GHz
    for the phase-D weight loads.

Sharding: batch 1024 -> 128 rows per core across 8 cores (data parallel),
as the sharding hint suggests; the all-reduce is the per-slot aggregated
evidence/precision, shrunk by the rank-4 trick.
"""

import numpy as np
from contextlib import ExitStack

import concourse.bass as bass
import concourse.tile as tile
from concourse import bacc, mybir
from concourse.bass_utils import run_bass_kernel_spmd

# problem dims (hardcoded per spec)
B, S, M, K, V, E, NQ, C = 1024, 512, 64, 64, 128, 64, 10000, 4
NCORES = 8
BL = B // NCORES            # 128 batch rows per core
TC = 32                     # timestep chunk
NCH = S // TC               # 16 chunks
NIDX = BL * TC              # gather indices per chunk = 4096
SPLIT = 4                   # sub-gathers per chunk (queue spreading)
NQUEUES = 4                 # SWDGE queues (max 4)
ARSPLIT = 12                # chunks in phase C/D's first (overlapped) part
AR_GROUP_ENDS = (4, 8, 12, 14, 15, 16)  # stat-collective group boundaries
F32 = mybir.dt.float32
F16 = mybir.dt.float16
I16 = mybir.dt.int16
GSCALE = 1024.0            # fp16 pre-scale for the tiny g values
ALU = mybir.AluOpType
AXT = mybir.AxisListType
ACTF = mybir.ActivationFunctionType

_CACHE = {}


def _build(single_core=False):
    nc = bacc.Bacc("TRN2", target_bir_lowering=False, debug=False,
                   num_devices=1 if single_core else NCORES,
                   num_swdge_queues=NQUEUES)

    t_simtab = nc.dram_tensor("simtab", [NQ + 1, M], F32, kind="ExternalInput")
    t_idx = nc.dram_tensor("idx16", [128, NCH * NIDX // 16], I16,
                           kind="ExternalInput")
    t_ftab = nc.dram_tensor("ftab", [BL, S, 3], F16, kind="ExternalInput")
    t_ebblk = nc.dram_tensor("ebblk", [128, S], F16, kind="ExternalInput")
    t_ebts = nc.dram_tensor("ebts", [M, S, 2], F16, kind="ExternalInput")
    t_sc = nc.dram_tensor("scal", [M, 8], F32, kind="ExternalInput")
    t_pb = nc.dram_tensor("pb", [BL, 1], F32, kind="ExternalInput")
    t_ident = nc.dram_tensor("ident", [128, 128], F16, kind="ExternalInput")
    t_dupm = nc.dram_tensor("dupm", [128, 128], F16, kind="ExternalInput")
    t_preds = nc.dram_tensor("preds", [BL, S], F32, kind="ExternalOutput")

    with tile.TileContext(nc) as tc:
        _build_body(nc, tc, single_core, t_simtab, t_idx, t_ftab, t_ebblk,
                    t_ebts, t_sc, t_pb, t_ident, t_dupm, t_preds)
    nc.compile()
    return nc


def _build_body(nc, tc, single_core, t_simtab, t_idx, t_ftab, t_ebblk,
                t_ebts, t_sc, t_pb, t_ident, t_dupm, t_preds):
    with ExitStack() as ctx:
        cpool = ctx.enter_context(tc.tile_pool(name="const", bufs=1))
        dpool = ctx.enter_context(tc.tile_pool(name="dram", bufs=1,
                                               space="DRAM"))
        # internal DRAM: grouped stat buffers.  The CC stream costs
        # ~4.3us/op + 0.69us per 2048-element slice, so the AllReduce is
        # split into a few groups fired as their chunks complete; only the
        # last (small) group is exposed after the gather phase.
        garr = [0] + list(AR_GROUP_ENDS)
        d_hin = [dpool.tile([M, (garr[i + 1] - garr[i]) * TC * 3], F16,
                            name=f"d_hin{i}")
                 for i in range(len(AR_GROUP_ENDS))]
        d_hout = [dpool.tile([M, (garr[i + 1] - garr[i]) * TC * 3], F16,
                             addr_space="Shared", name=f"d_hout{i}")
                  for i in range(len(AR_GROUP_ENDS))]

        # resident SBUF (idx in two pieces so the first gather starts early)
        idx_sb = cpool.tile([128, NCH * NIDX // 16], I16)
        nc.sync.dma_start(idx_sb[:, 0:NIDX // 16],
                          t_idx.ap()[:, 0:NIDX // 16])
        nc.sync.dma_start(idx_sb[:, NIDX // 16:],
                          t_idx.ap()[:, NIDX // 16:])
        ftab_sb = cpool.tile([BL, S, 3], F16)
        nc.sync.dma_start(ftab_sb[:], t_ftab.ap())
        sc_sb = cpool.tile([M, 8], F32)
        nc.sync.dma_start(sc_sb[:], t_sc.ap())
        pb_sb = cpool.tile([BL, 1], F32)
        nc.sync.dma_start(pb_sb[:], t_pb.ap())
        ident_sb = cpool.tile([128, 128], F16)
        nc.sync.dma_start(ident_sb[:], t_ident.ap())
        dupm_sb = cpool.tile([128, 128], F16)
        nc.sync.dma_start(dupm_sb[:], t_dupm.ap())
        ebblk_sb = cpool.tile([128, S], F16)
        nc.sync.dma_start(ebblk_sb[:], t_ebblk.ap())
        ebts_sb = cpool.tile([M, S, 2], F16)
        nc.sync.dma_start(ebts_sb[:], t_ebts.ap())
        zr = cpool.tile([BL, S], F32)
        eet = cpool.tile([128, S // 2, 128], F16)  # (t%2)*64+m, t//2, b

        hs = cpool.tile([M, S, 3], F16)

        def fire_ar(g):
            """AllReduce stat group g and load the result into hs."""
            lo, hi = garr[g], garr[g + 1]
            if single_core:
                nc.sync.dma_start(d_hout[g][:], d_hin[g][:])
            else:
                nc.gpsimd.collective_compute(
                    "AllReduce", ALU.add,
                    replica_groups=[list(range(NCORES))],
                    ins=[d_hin[g][:].opt()],
                    outs=[d_hout[g][:].opt()],
                )
            nc.sync.dma_start(
                hs[:, lo * TC:hi * TC, :],
                d_hout[g][:].rearrange("m (s k) -> m s k", k=3))

        # ---------------- phase A: per-chunk softmax stats + H matmuls
        actx = ctx.enter_context(ExitStack())
        apool = actx.enter_context(tc.tile_pool(name="pha", bufs=4))
        bpool = actx.enter_context(tc.tile_pool(name="phb", bufs=2))
        epool = actx.enter_context(tc.tile_pool(name="phe", bufs=3))
        spool = actx.enter_context(tc.tile_pool(name="phs", bufs=2))
        pspool = actx.enter_context(
            tc.tile_pool(name="php", bufs=2, space="PSUM"))
        tpool = actx.enter_context(
            tc.tile_pool(name="pht", bufs=2, space="PSUM"))
        zpool = actx.enter_context(
            tc.tile_pool(name="phz", bufs=2, space="PSUM"))
        qctr = 0
        for c in range(NCH):
            ts = slice(c * TC, (c + 1) * TC)
            ge = apool.tile([BL, TC * M], F32, tag="ge")
            ge3 = ge[:].rearrange("p (a b) -> p a b", b=M)
            nsub = NIDX // SPLIT
            tsub = TC // SPLIT
            for a in range(SPLIT):
                i0 = c * NIDX // 16 + a * nsub // 16
                nc.gpsimd.dma_gather(
                    out_ap=ge3[:, a * tsub:(a + 1) * tsub, :],
                    in_ap=t_simtab.ap(),
                    idxs_ap=idx_sb[:, i0:i0 + nsub // 16],
                    num_idxs=nsub,
                    num_idxs_reg=nsub,
                    elem_size=M,
                    single_packet=False,
                    queue_num=qctr % NQUEUES,
                )
                qctr += 1
            # exp(sim) straight to fp16; the per-(t,m) bias factor eb is NOT
            # multiplied in here — it commutes out of every b-contraction, so
            # it is applied to the tiny [M,S]-sized stats (ebts), the softmax
            # denominator (via the ebblk matmul columns) and g (in phase C)
            ge16 = epool.tile([BL, TC * M], F16, tag="ge16")
            nc.scalar.activation(ge16[:], ge[:], ACTF.Exp)
            ee3 = ge16[:].rearrange("p (a b) -> p a b", b=M)
            # transpose into the resident read-dot operand via the PE
            # (dma_start_transpose starves the gather SDMA queues), then
            # drain PSUM->SBUF on the scalar engine to keep the DVE light
            pst = tpool.tile([128, TC // 2, 128], F16, tag="pst")
            eeb = ge16[:].rearrange("p (k b) -> p k b", b=128)
            for kk in range(TC // 2):
                nc.tensor.transpose(pst[:, kk, :], eeb[:, kk, :], ident_sb[:])
            nc.scalar.activation(
                eet[:, c * TC // 2:(c + 1) * TC // 2, :], pst[:], ACTF.Copy)
            # softmax denominator Z[b,t] = sum_m ge*eb via PE pair-matmuls
            # against the block-diagonal eb columns
            psZ = zpool.tile([BL, TC], F32, tag="psZ")
            for jj in range(TC // 2):
                j = c * TC // 2 + jj
                nc.tensor.matmul(psZ[:, 2 * jj:2 * jj + 2],
                                 lhsT=eet[:, j, :],
                                 rhs=ebblk_sb[:, 2 * j:2 * j + 2],
                                 start=True, stop=True)
            zrc = zr[:, ts]
            nc.vector.reciprocal(zrc, psZ[:])
            zr2 = spool.tile([BL, TC], F32, tag="zr2")
            nc.vector.tensor_tensor(zr2[:], zrc, zrc, ALU.mult)
            fp = spool.tile([BL, TC, 3], F16, tag="fp")
            nc.vector.tensor_tensor(
                fp[:, :, 0:2], ftab_sb[:, ts, 0:2],
                zrc.unsqueeze(2).broadcast_to([BL, TC, 2]), ALU.mult)
            nc.vector.tensor_tensor(
                fp[:, :, 2:3], ftab_sb[:, ts, 2:3],
                zr2[:].unsqueeze(2).broadcast_to([BL, TC, 1]), ALU.mult)
            e2 = bpool.tile([BL, TC * M], F16, tag="e2")
            nc.scalar.activation(e2[:], ge16[:], ACTF.Square)
            e23 = e2[:].rearrange("p (a b) -> p a b", b=M)
            hp = pspool.tile([M, TC * 3], F32, tag="hp")
            for t in range(TC):
                nc.tensor.matmul(hp[:, t * 3:t * 3 + 2], lhsT=ee3[:, t, :],
                                 rhs=fp[:, t, 0:2], start=True, stop=True)
                nc.tensor.matmul(hp[:, t * 3 + 2:t * 3 + 3],
                                 lhsT=e23[:, t, :], rhs=fp[:, t, 2:3],
                                 start=True, stop=True)
            # apply the commuted eb / eb^2 factors while draining PSUM
            hbc = spool.tile([M, TC * 3], F16, tag="hbc")
            hb3 = hbc[:].rearrange("m (s k) -> m s k", k=3)
            hp3 = hp[:].rearrange("m (s k) -> m s k", k=3)
            nc.vector.tensor_tensor(
                hb3[:, :, 0:2], hp3[:, :, 0:2],
                ebts_sb[:, ts, 0:1].broadcast_to([M, TC, 2]), ALU.mult)
            nc.vector.tensor_tensor(
                hb3[:, :, 2:3], hp3[:, :, 2:3],
                ebts_sb[:, ts, 1:2].broadcast_to([M, TC, 1]), ALU.mult)
            gidx = next(i for i in range(len(AR_GROUP_ENDS))
                        if c < garr[i + 1])
            off = (c - garr[gidx]) * TC * 3
            nc.scalar.dma_start(d_hin[gidx][:, off:off + TC * 3], hbc[:])
            # fire each group's AllReduce 3 chunks after its last spill, so
            # the trigger's sem wait never blocks the gather descriptor
            # stream on the gpsimd queue
            for g in range(len(AR_GROUP_ENDS)):
                if c == garr[g + 1] - 1 + 3:
                    fire_ar(g)
        for g in range(len(AR_GROUP_ENDS)):
            if garr[g + 1] - 1 + 3 > NCH - 1:
                fire_ar(g)
        actx.close()

        TA = ARSPLIT * TC            # phase C/D split point
        NGA = TA // 128              # phase-D groups fully inside part A
        NG = S // 128

        # ---------------- phase C: cumsums + read vector g  (all [M, S]),
        # split at TA so the part-A compute overlaps AllReduce B.  carry[:, k]
        # holds each scan's part-A total for the part-B fix-up.
        cpool2 = ctx.enter_context(tc.tile_pool(name="phc", bufs=1))
        css = cpool2.tile([M, S], F32)
        rcs = cpool2.tile([M, S], F32)
        cc = cpool2.tile([M, S], F32)
        sfac = cpool2.tile([M, S], F32)
        num = cpool2.tile([M, S], F32)
        den = cpool2.tile([M, S], F32)
        g16 = cpool2.tile([128, S], F16)
        carry = cpool2.tile([M, 8], F32)
        gblk = cpool.tile([128, S], F16)
        nc.vector.memset(gblk[:], 0.0)
        nc.vector.memset(g16[M:128, :], 0.0)
        pdup = ctx.enter_context(
            tc.tile_pool(name="phgd", bufs=1, space="PSUM"))

        def phase_c(lo, hi, first):
            sl = slice(lo, hi)
            n = hi - lo
            tag = "A" if first else "B"
            nc.vector.tensor_scalar_add(css[:, sl], hs[:, sl, 0], 1e-8)
            nc.vector.reciprocal(rcs[:, sl], css[:, sl])
            nc.vector.tensor_tensor(cc[:, sl], hs[:, sl, 1], hs[:, sl, 0],
                                    ALU.mult)
            nc.vector.tensor_tensor(cc[:, sl], cc[:, sl], rcs[:, sl],
                                    ALU.mult)
            nc.vector.tensor_scalar(cc[:, sl], cc[:, sl], 1.0 / B, None,
                                    ALU.mult)
            nc.vector.tensor_tensor(sfac[:, sl], cc[:, sl], rcs[:, sl],
                                    ALU.mult)
            for k in range(2):
                # k == 0: the gp-weighted evidence cumsum (num);
                # k == 1: the precision cumsum over cc (den)
                src = cc[:, sl] if k == 1 else None
                ch = cpool2.tile([M, n + 1], F32, tag="ch" + tag)
                if k == 0:
                    hsk = cpool2.tile([M, n], F32, tag="hsk" + tag)
                    nc.vector.tensor_tensor(hsk[:], hs[:, sl, 2],
                                            sfac[:, sl], ALU.mult)
                    src = hsk[:]
                nc.vector.memset(ch[:, 0:1], 0.0)
                nc.vector.tensor_tensor_scan(ch[:, 1:n + 1], src, src, 0.0,
                                             ALU.add, ALU.bypass)
                if first:
                    nc.vector.tensor_copy(carry[:, k:k + 1], ch[:, n:n + 1])
                else:
                    nc.vector.tensor_scalar_add(ch[:, 0:n], ch[:, 0:n],
                                                carry[:, k:k + 1])
                if k == 0:
                    nc.vector.tensor_scalar_add(num[:, sl], ch[:, 0:n],
                                                sc_sb[:, 5:6])
                else:
                    nc.vector.tensor_scalar_add(den[:, sl], ch[:, 0:n],
                                                sc_sb[:, 4:5])
            # g = num / den, times the commuted eb factor, scaled into the
            # fp16 normal range
            nc.vector.reciprocal(den[:, sl], den[:, sl])
            nc.vector.tensor_tensor(num[:, sl], num[:, sl], den[:, sl],
                                    ALU.mult)
            nc.vector.tensor_tensor(num[:, sl], num[:, sl],
                                    ebts_sb[:, sl, 0], ALU.mult)
            nc.vector.tensor_scalar(g16[0:M, sl], num[:, sl], GSCALE, None,
                                    ALU.mult)
            # duplicate g16 onto both partition halves with one PE matmul
            # (dupm[p, i] = 1 iff p == i mod 64), then build the
            # block-diagonal column pairs: col 2j keeps only the even-t
            # (top) half, col 2j+1 only the odd-t (bottom) half
            psdup = pdup.tile([128, ARSPLIT * TC], F32, tag="psdup")
            nc.tensor.matmul(psdup[:, 0:hi - lo], lhsT=dupm_sb[:],
                             rhs=g16[:, sl], start=True, stop=True)
            pd2 = psdup[:, 0:hi - lo].rearrange("p (j two) -> p j two", two=2)
            gb2 = gblk[:, sl].rearrange("p (j two) -> p j two", two=2)
            nc.vector.tensor_copy(gb2[0:M, :, 0], pd2[0:M, :, 0])
            nc.vector.tensor_copy(gb2[M:128, :, 1], pd2[M:128, :, 1])

        # ---------------- phase D: dot[b, 2j:2j+2] = eet_pair.T @ gblk_pair
        # (the block-diagonal gblk columns keep even/odd t separate), landing
        # directly in [b, t] layout; preds = zr/GSCALE * dot + pred_b
        rtile = cpool.tile([BL, S], F32)
        dpool2 = ctx.enter_context(tc.tile_pool(name="phd", bufs=2))
        psd = ctx.enter_context(
            tc.tile_pool(name="phdp", bufs=4, space="PSUM"))
        wpool = ctx.enter_context(
            tc.tile_pool(name="phw", bufs=1, space="PSUM"))
        psDs = {}

        def phase_d_mm(glo, ghi):
            for gi in range(glo, ghi):
                psD = psd.tile([BL, 128], F32, tag="psD")
                psDs[gi] = psD
                for jj in range(64):
                    j = gi * 64 + jj
                    nc.tensor.matmul(psD[:, 2 * jj:2 * jj + 2],
                                     lhsT=eet[:, j, :],
                                     rhs=gblk[:, 2 * j:2 * j + 2],
                                     start=True, stop=True)

        def phase_d_fin(glo, ghi):
            for gi in range(glo, ghi):
                gsl = slice(gi * 128, (gi + 1) * 128)
                rt32 = dpool2.tile([BL, 128], F32, tag="rt32")
                nc.vector.tensor_tensor(rt32[:], psDs[gi][:], zr[:, gsl],
                                        ALU.mult)
                nc.vector.tensor_scalar(rtile[:, gsl], rt32[:], 1.0 / GSCALE,
                                        pb_sb[:, 0:1], ALU.mult, ALU.add)

        phase_c(0, TA, True)
        phase_d_mm(0, NGA)
        phase_d_fin(0, NGA)
        # keep the PE's HAM clock warm across the gap between the two
        # phase-D halves (waiting on the last stat collectives); anchored on
        # the last eet chunk so these fill the gap rather than running early
        warm = wpool.tile([128, 128], F16)
        for _ in range(96):
            nc.tensor.transpose(warm[:], eet[:, S // 2 - 1, :], ident_sb[:])
        phase_c(TA, S, False)
        phase_d_mm(NGA, NG)
        phase_d_fin(NGA, NG)
        nc.sync.dma_start(t_preds.ap(), rtile[:])


def _softplus(x):
    return np.logaddexp(0.0, x)


def _host_prep(inputs):
    """All the cheap host-side precomputation; returns per-core in_maps."""
    q_ids = np.asarray(inputs["q_ids"], np.int64)          # [B, S]
    responses = np.asarray(inputs["responses"], np.int64)  # [B, S]
    q_table = np.asarray(inputs["q_table"], np.float32)
    key_embeds = np.asarray(inputs["key_embeds"], np.float32)
    alpha_mean = np.asarray(inputs["alpha_mean"], np.float32)
    alpha_log_var = np.asarray(inputs["alpha_log_var"], np.float32)
    beta_base = np.asarray(inputs["beta_base"], np.float32)
    beta_offsets = np.asarray(inputs["beta_offsets"], np.float32)
    theta_mean0 = np.asarray(inputs["theta_mean0"], np.float32)
    theta_log_var0 = np.asarray(inputs["theta_log_var0"], np.float32)
    q2k_w = np.asarray(inputs["q2k_w"], np.float32)
    q2k_b = np.asarray(inputs["q2k_b"], np.float32)
    qa_w = np.asarray(inputs["qa_w"], np.float32)
    qa_b = np.asarray(inputs["qa_b"], np.float32)
    qae_w = np.asarray(inputs["qae_w"], np.float32)
    qae_b = np.asarray(inputs["qae_b"], np.float32)
    pred_w = np.asarray(inputs["pred_w"], np.float32)
    pred_b = np.asarray(inputs["pred_b"], np.float32)
    alpha_noise = np.asarray(inputs["alpha_noise"], np.float32)
    beta_noise = np.asarray(inputs["beta_noise"], np.float32)

    # sim table: folds q_table @ q2k_w @ key_embeds.T (+ bias) into a gather
    w_qm = q2k_w @ key_embeds.T                            # [E, M]
    b_m = q2k_b @ key_embeds.T                             # [M]
    simtab = (q_table @ w_qm + b_m[None]).astype(np.float32)

    # per-(t, m) logit bias -> eb = exp(bias)
    alpha = np.exp(alpha_mean[None] + alpha_noise
                   * np.exp(0.5 * alpha_log_var)[None])    # [S, M]
    base = beta_base[None] + beta_noise * 0.1              # [S, M]
    offs = _softplus(beta_offsets)                         # [M, C-1]
    cum = np.concatenate([np.zeros((M, 1), np.float32),
                          np.cumsum(offs, 1)[:, :C - 2]], 1)
    beta_mean = base + cum.mean(1)[None]
    diff_sim = np.exp(-0.5 * beta_mean ** 2)
    ebt = np.exp(0.3 * alpha + 0.2 * diff_sim).astype(np.float32)  # [S, M]
    # block-diagonal eb columns for the on-device Z matmuls: col 2j keeps
    # the even-t value on the top partition half, col 2j+1 the odd-t value
    # on the bottom half
    ebblk = np.zeros((128, S), np.float16)
    ebblk[0:M, 0::2] = ebt.T[:, 0::2]
    ebblk[M:128, 1::2] = ebt.T[:, 1::2]
    # eb and eb^2 per (m, t) for the commuted H-stat scaling
    ebts = np.empty((M, S, 2), np.float16)
    ebts[:, :, 0] = ebt.T
    ebts[:, :, 1] = (ebt.T ** 2)

    # evidence scalars per (b, t)
    rn = responses.astype(np.float32) / (C - 1)
    p = np.clip(rn, 0.01, 0.99)
    ae = np.log(p) - np.log1p(-p)
    pr = 0.5 + np.abs(rn - 0.5) * 2.0
    q01 = q_ids.astype(np.float32) / NQ

    # rank-4 decomposition of comb over V
    w0v = qa_w[0] @ qae_w
    w1v = qa_w[1] @ qae_w
    bv = qa_b @ qae_w + qae_b
    pw = pred_w[:, 0]
    gp = 0.5 * np.array([w0v @ pw, w1v @ pw, bv @ pw, pw.sum()], np.float32)

    alo = np.exp(-theta_log_var0[:, 0])                    # [M]
    n0pw = alo * (theta_mean0 @ pw)                        # [M]
    sc = np.zeros((M, 8), np.float32)
    sc[:, 0:4] = gp[None, :]
    sc[:, 4] = alo
    sc[:, 5] = n0pw

    pb = np.full((BL, 1), float(pred_b[0]), np.float32)
    ident = np.eye(128, dtype=np.float16)
    dupm = np.zeros((128, 128), np.float16)
    dupm[np.arange(64), np.arange(64)] = 1.0
    dupm[np.arange(64), np.arange(64) + 64] = 1.0

    in_maps = []
    for core in range(NCORES):
        bs = slice(core * BL, (core + 1) * BL)
        qs = q_ids[bs]                                     # [128, S]
        # gather indices, chunk-major, wrapped in 16 partitions
        blocks = []
        for c in range(NCH):
            flat = qs[:, c * TC:(c + 1) * TC].T.reshape(-1)  # t-major
            w16 = flat.reshape(NIDX // 16, 16).T             # [16, NIDX/16]
            blocks.append(np.tile(w16, (8, 1)))
        idx16 = np.concatenate(blocks, axis=1).astype(np.int16)

        # the four evidence features only ever enter through the fixed
        # combination sum_k gp_k * f_k (gp commutes through the cumsum),
        # so fold them into a single column on the host
        ftab = np.empty((BL, S, 3), np.float16)
        ftab[:, :, 0] = 1.0
        ftab[:, :, 1] = pr[bs]
        ftab[:, :, 2] = (gp[0] * q01[bs] + gp[1] * rn[bs] + gp[2]
                         + gp[3] * ae[bs])

        in_maps.append({
            "simtab": simtab,
            "idx16": idx16,
            "ftab": ftab,
            "ebblk": ebblk,
            "ebts": ebts,
            "scal": sc,
            "pb": pb,
            "ident": ident,
            "dupm": dupm,
        })
    return in_maps


def _run(in_maps, **kw):
    if "nc" not in _CACHE:
        _CACHE["nc"] = _build()
    res = run_bass_kernel_spmd(_CACHE["nc"], in_maps,
                               core_ids=list(range(NCORES)), **kw)
    preds = np.concatenate([res.results[c]["preds"] for c in range(NCORES)],
                           axis=0)
    return preds.astype(np.float32), res


def kernel(**inputs) -> np.ndarray:
    return _run(_host_prep(inputs))[0]


if __name__ == "__main__":
    pass
